# revision 1
# baseline (speedup 1.0000x reference)
"""Bahdanau attention TRN2 kernel (B=8 data-parallel over 8 NeuronCores).

Instead of materializing tanh(q_iu + k_ju) over the full [Tq, Tv, U] cube
(33.5M ACT tanh evals/core -- the old bottleneck at ~225us), we use a
separable Fourier expansion of the ridge kernel (fitted offline, tied pairs):

  tanh(a+b) ~= sum_k c_k [ sin(w_k a) cos(w_k b) + cos(w_k a) sin(w_k b) ]
             (6 frequencies, 12 rank-1 terms, end-to-end rel err ~2e-3)

so the score tensor becomes 12 PE matmuls contracting over u per j-chunk:

  scores[j,i] = sum_u scale_u * tanh(q[i,u]+k[j,u])
             ~= sum_m [scaletab_um * g_m(kT)]^T @ [f_m(qT)]

Per core (u=128 on partitions everywhere in the hot path):
  qT[u,i] = Wa^T @ queryT-chunks, kT likewise      (PE, f32r full-rate)
  ACT Sin spline is only valid for |x| <= ~pi, so per frequency (one shared
  chain for the sin AND cos atom): t0 = qT*w/2pi (DVE); r = round(t0) (DVE
  cast->i32 rounds to nearest); d = t0 - r in [-.5,.5] (DVE tensor_tensor);
  e = |d| (DVE). Then batched ACT: sin atoms = Sin(2pi*d), cos atoms =
  Sin(pi/2 - 2pi*e) = cos(2pi*d). Lowest frequency skips reduction (|arg|<pi).
  wk_m = t_m * scaletab[:,m]                       (DVE single-op mul)
  sT[j,i] += wk_m[:,jc-chunk]^T @ tq_m             (PE, fp16, PSUM accum)
  wmT = Exp(sT + maskbias_j - 6)                   (ACT, per-partition bias;
      mask folded into the bias, -6 shift keeps fp16 exp in range)
  ctx[i,d] = wmT^T @ value16; Z[i] = wmT^T @ ones  (PE; Z via N=1 matmuls in
      4 separate PSUM banks -- interleaved accum groups in one bank drop data)
  out = ctx * (1/Z)                                (ACT Copy w/ per-part scale)
No GPSIMD anywhere: concurrent GPSIMD traffic contends for SBUF ports and
inflates DVE op durations 3-10x (measured).
Host-side prep is layout-only (transposes, fp16 value, mask->bias floats,
scale_u * c_m table).
"""

import sys

if "/opt/trn_rl_repo" not in sys.path:
    sys.path.insert(0, "/opt/trn_rl_repo")

import math
import numpy as np

import concourse.bacc as bacc
import concourse.bass as bass
import concourse.tile as tile
import concourse.mybir as mybir

F32 = mybir.dt.float32
F32R = mybir.dt.float32r
F16 = mybir.dt.float16
I32 = mybir.dt.int32
AF = mybir.ActivationFunctionType
OP = mybir.AluOpType

B, TQ, TV, D, U = 8, 512, 512, 512, 128
N_CORES = 8
TWO_PI = 2 * math.pi
PI_2 = math.pi / 2
EXP_SHIFT = -6.0
MASK_NEG = -30.0
N_WARM = 8  # PE warmup matmuls (HAM un-throttle during DMA lead-in)

# Tied Fourier fit of tanh(a+b) (params_tied.npy): per frequency k the terms
#   2k   = c_sc * sin(wa_k a) * cos(wb_k b)
#   2k+1 = c_cs * cos(wa_k a) * sin(wb_k b)
FREQS = [
    # (wa, wb, c_sc, c_cs)
    (0.2680186991122955, 0.2680264886317569, 1.2391441309603777, 1.2391207334538665),
    (0.8154964955419217, 0.8155062117103967, 0.3406703579449399, 0.34066246167970216),
    (1.41854462474336, 1.4184776736885334, 0.1410391879211783, 0.14104266844075805),
    (2.1176004979963965, 2.117258028698721, 0.054562953796606264, 0.054562027169262395),
    (2.998236719290852, 2.998386748962967, 0.01914059438192951, 0.01914090881147914),
]
NF = len(FREQS)
R = 2 * NF
# frequency groups: group 0 = direct (no range reduction needed), rest chunked
GROUPS = [[0], [1, 2], [3, 4]]
DIRECT = {0}  # |w|*qmax/2pi < 0.5: d = t0, no rounding needed


def _emit(nc, debug=False, outer_repeat=1):
    queryT = nc.dram_tensor("queryT", [D, TQ], F16, kind="ExternalInput")
    keyT = nc.dram_tensor("keyT", [D, TV], F16, kind="ExternalInput")
    value16 = nc.dram_tensor("value16", [TV, D], F16, kind="ExternalInput")
    wua = nc.dram_tensor("wua", [2 * D, U], F16, kind="ExternalInput")
    ctab = nc.dram_tensor("ctab", [128, R + 6], F32, kind="ExternalInput")
    ctx = nc.dram_tensor("ctx", [TQ, D], F32, kind="ExternalOutput")
    env = dict(locals())
    if debug:
        env["dbg_qT"] = nc.dram_tensor("dbg_qT", [U, TQ], F32, kind="ExternalOutput")
        env["dbg_sT"] = nc.dram_tensor("dbg_sT", [128, 4, TQ], F32, kind="ExternalOutput")
        env["dbg_z"] = nc.dram_tensor("dbg_z", [128, 4], F32, kind="ExternalOutput")

    with tile.TileContext(nc) as tc:
        for _rep in range(outer_repeat):
            _emit_body(nc, tc, env, debug)


def _emit_body(nc, tc, env, debug):
    queryT, keyT, value16, wua = (
        env["queryT"], env["keyT"], env["value16"], env["wua"]
    )
    ctab, ctx = env["ctab"], env["ctx"]

    with tc.tile_pool(name="const", bufs=1) as const:
        ctab_sb = const.tile([128, R + 6], F32, name="ctab_sb")
        ones16 = const.tile([128, 1], F16, name="ones16")
        value_sb = const.tile([128, 4, D], F16, name="value_sb")
        qT16 = const.tile([U, TQ], F16, name="qT16")
        kT16 = const.tile([U, TV], F16, name="kT16")
        wmT_sb = const.tile([128, 4, TQ], F16, name="wmT_sb")
        z_sb = const.tile([128, 4], F32, name="z_sb")
        zr_sb = const.tile([128, 4], F32, name="zr_sb")
        octx_sb = const.tile([128, 4, D], F32, name="octx_sb")

        nc.vector.memset(ones16[:], 1.0)

        # ---- PE warmup during the DMA lead-in (HAM un-throttle). wsrc
        # lives in the const pool so its SBUF slot is never recycled into a
        # DMA-target tile (that reuse made the qin DMA wait on the warm MMs);
        # memset on GPSIMD, whose preamble finishes earliest.
        wsrc = const.tile([128, 512], F16, name="wsrc")
        nc.gpsimd.memset(wsrc[:], 0.0)
        # tiny dummy Sin: pin the first ACT table load (sin set) into the
        # DMA lead-in window; qT16/kT16 Copy then rides the same set
        nc.scalar.activation(z_sb[:, 1:2], wsrc[:, 0:1], AF.Sin, bias=0.0)
        with tc.tile_pool(name="warmps", bufs=1, space="PSUM") as warmps:
            wps = warmps.tile([128, 512], F32, name="wps")
            for _ in range(N_WARM):
                nc.tensor.matmul(wps[:], wsrc[:, :128], wsrc[:])

        # ---- projections: qT = Wa^T @ queryT-chunks, kT likewise ----
        with (
            tc.tile_pool(name="projin", bufs=1) as projin,
            tc.tile_pool(name="projps", bufs=1, space="PSUM") as projps,
        ):
            qin = projin.tile([128, 4, TQ], F16, name="qin")
            kin = projin.tile([128, 4, TV], F16, name="kin")
            wua_sb = projin.tile([128, 8, U], F16, name="wua_sb")
            qT_r = queryT.ap().rearrange("(c p) i -> p c i", p=128)
            kT_r = keyT.ap().rearrange("(c p) i -> p c i", p=128)
            wua_r = wua.ap().rearrange("(s c p) u -> p (s c) u", p=128, c=4)
            nc.sync.dma_start(out=wua_sb[:], in_=wua_r)
            nc.sync.dma_start(out=kin[:], in_=kT_r)
            nc.sync.dma_start(out=qin[:], in_=qT_r)
            nc.sync.dma_start(out=ctab_sb[:], in_=ctab.ap())
            nc.sync.dma_start(
                out=value_sb[:],
                in_=value16.ap().rearrange("(c p) d -> p c d", p=128),
            )
            qT_ps = projps.tile([U, TQ], F32, name="qT_ps")
            kT_ps = projps.tile([U, TV], F32, name="kT_ps")
            for dc in range(4):
                nc.tensor.matmul(
                    kT_ps[:], wua_sb[:, 4 + dc, :], kin[:, dc, :],
                    start=(dc == 0), stop=(dc == 3),
                )
            nc.vector.tensor_copy(out=kT16[:], in_=kT_ps[:])
            for dc in range(4):
                nc.tensor.matmul(
                    qT_ps[:], wua_sb[:, dc, :], qin[:, dc, :],
                    start=(dc == 0), stop=(dc == 3),
                )
            nc.vector.tensor_copy(out=qT16[:], in_=qT_ps[:])

        # ---- main loop over frequency groups ----
        with tc.tile_pool(name="spsum", bufs=1, space="PSUM") as spsum:
            sT_ps = [
                spsum.tile([128, TQ], F32, name=f"sT_ps{jc}") for jc in range(4)
            ]
            with (
                tc.tile_pool(name="t0pool", bufs=3) as t0pool,
                tc.tile_pool(name="rpool", bufs=3) as rpool,
                tc.tile_pool(name="dpool", bufs=3) as dpool,
                tc.tile_pool(name="tpool", bufs=3) as tpool,
                tc.tile_pool(name="wkpool", bufs=3) as wkpool,
            ):
                # (side, kf) -> (tile, sin_slot, cos_slot)
                atom = {}
                for gi, kfs in enumerate(GROUPS):
                    n = len(kfs)
                    for side in ("k", "q"):
                        src = qT16 if side == "q" else kT16
                        wf = 0 if side == "q" else 1
                        tt = tpool.tile([128, 2 * n * TQ], F16,
                                        name=f"t{side}", tag=f"t{side}")
                        if all(kf in DIRECT for kf in kfs):
                            # short chain: one ACT per atom straight from src
                            for j, kf in enumerate(kfs):
                                w = FREQS[kf][wf]
                                nc.scalar.activation(
                                    tt[:, j * TQ : (j + 1) * TQ], src[:],
                                    AF.Sin, bias=ctab_sb[:, R + 4 : R + 5], scale=w,
                                )
                                nc.scalar.activation(
                                    tt[:, (n + j) * TQ : (n + j + 1) * TQ],
                                    src[:],
                                    AF.Sin, bias=ctab_sb[:, R + 5 : R + 6], scale=w,
                                )
                            for j, kf in enumerate(kfs):
                                atom[(side, kf)] = (tt, j, n + j)
                            if side != "k":
                                continue
                            wk = wkpool.tile([128, 2 * n * TV], F16,
                                             name="wk", tag="wk")
                            wk_slot = {}
                            for j, kf in enumerate(kfs):
                                ktile, ksin, kcos = atom[("k", kf)]
                                for t_loc, (term, kslot) in enumerate(
                                    [(2 * kf, kcos), (2 * kf + 1, ksin)]
                                ):
                                    sl = 2 * j + t_loc
                                    nc.vector.tensor_scalar_mul(
                                        wk[:, sl * TV : (sl + 1) * TV],
                                        ktile[:, kslot * TV : (kslot + 1) * TV],
                                        ctab_sb[:, term : term + 1],
                                    )
                                    wk_slot[term] = sl
                            continue
                        dt_ = dpool.tile([128, 2 * n * TQ], F16,
                                         name=f"d{side}", tag=f"d{side}")
                        for j, kf in enumerate(kfs):
                            w2p = FREQS[kf][wf] / TWO_PI
                            dsl = dt_[:, j * TQ : (j + 1) * TQ]
                            if kf in DIRECT:
                                # |t0| < 0.5 always: d = t0, no rounding
                                nc.vector.tensor_scalar_mul(dsl, src[:], w2p)
                            else:
                                t0 = t0pool.tile([128, TQ], F32, name="t0",
                                                 tag="t0")
                                nc.vector.tensor_scalar_mul(t0[:], src[:], w2p)
                                r = rpool.tile([128, TQ], I32, name="r", tag="r")
                                nc.vector.tensor_copy(out=r[:], in_=t0[:])
                                nc.vector.tensor_tensor(
                                    out=dsl, in0=t0[:], in1=r[:], op=OP.subtract,
                                )
                            # |d| for the cos atom on DVE: max(-d, d)
                            nc.vector.scalar_tensor_tensor(
                                out=dt_[:, (n + j) * TQ : (n + j + 1) * TQ],
                                in0=dsl, scalar=-1.0, in1=dsl,
                                op0=OP.mult, op1=OP.max,
                            )
                        nc.scalar.activation(
                            tt[:, : n * TQ], dt_[:, : n * TQ],
                            AF.Sin, bias=ctab_sb[:, R + 4 : R + 5], scale=TWO_PI,
                        )
                        nc.scalar.activation(
                            tt[:, n * TQ :], dt_[:, n * TQ :],
                            AF.Sin, bias=ctab_sb[:, R + 5 : R + 6], scale=-TWO_PI,
                        )
                        for j, kf in enumerate(kfs):
                            atom[(side, kf)] = (tt, j, n + j)
                        last_tt = tt
                        if side == "k":
                            # folds: term 2k uses k-side cos, 2k+1 k-side sin
                            wk = wkpool.tile([128, 2 * n * TV], F16,
                                             name="wk", tag="wk")
                            wk_slot = {}
                            for j, kf in enumerate(kfs):
                                ktile, ksin, kcos = atom[("k", kf)]
                                for t_loc, (term, kslot) in enumerate(
                                    [(2 * kf, kcos), (2 * kf + 1, ksin)]
                                ):
                                    sl = 2 * j + t_loc
                                    nc.vector.tensor_scalar_mul(
                                        wk[:, sl * TV : (sl + 1) * TV],
                                        ktile[:, kslot * TV : (kslot + 1) * TV],
                                        ctab_sb[:, term : term + 1],
                                    )
                                    wk_slot[term] = sl
                    # score matmuls (PSUM accumulate across all R terms)
                    for j, kf in enumerate(kfs):
                        qtile, qsin, qcos = atom[("q", kf)]
                        for term, qslot in ((2 * kf, qsin), (2 * kf + 1, qcos)):
                            sl = wk_slot[term]
                            for jc in range(4):
                                nc.tensor.matmul(
                                    sT_ps[jc][:],
                                    wk[:, sl * TV + jc * 128 : sl * TV + (jc + 1) * 128],
                                    qtile[:, qslot * TQ : (qslot + 1) * TQ],
                                    start=(term == 0), stop=(term == R - 1),
                                )
            # tiny dummy Exp first: pulls the exp table load off the
            # critical path (overlaps the last group's score matmuls). It
            # reads the last Sin output tile so the scheduler cannot hoist
            # it above the Sin ops (which would thrash the table sets).
            nc.scalar.activation(
                z_sb[:, 0:1], last_tt[:, 0:1], AF.Exp, bias=0.0
            )
            # exp with mask+shift folded into the per-partition bias
            for jc in range(4):
                nc.scalar.activation(
                    wmT_sb[:, jc, :], sT_ps[jc][:], AF.Exp,
                    bias=ctab_sb[:, R + jc : R + jc + 1],
                )
            if debug:
                for jc in range(4):
                    nc.sync.dma_start(
                        out=env["dbg_sT"].ap()[:, jc, :], in_=sT_ps[jc][:]
                    )

        # ---- tail: ctx = wmT^T @ value, Z = wmT^T @ ones, normalize ----
        with (
            tc.tile_pool(name="cpsum", bufs=1, space="PSUM") as cpsum,
            tc.tile_pool(name="zpsum", bufs=1, space="PSUM") as zpsum,
        ):
            c_ps = [cpsum.tile([128, D], F32, name=f"c_ps{ic}") for ic in range(4)]
            z_ps = [zpsum.tile([128, 1], F32, name=f"z_ps{ic}") for ic in range(4)]
            # ic-outer so c_ps[0] completes early and its normalize + DMA-out
            # overlap the remaining ic's matmuls
            for ic in range(4):
                for jc in range(4):
                    lhsT = wmT_sb[:, jc, ic * 128 : (ic + 1) * 128]
                    nc.tensor.matmul(
                        c_ps[ic][:], lhsT, value_sb[:, jc, :],
                        start=(jc == 0), stop=(jc == 3),
                    )
                    nc.tensor.matmul(
                        z_ps[ic][:], lhsT, ones16[:],
                        start=(jc == 0), stop=(jc == 3),
                    )
                nc.vector.tensor_copy(out=z_sb[:, ic : ic + 1], in_=z_ps[ic][:])
                nc.vector.reciprocal(
                    out=zr_sb[:, ic : ic + 1], in_=z_sb[:, ic : ic + 1]
                )
                nc.scalar.activation(
                    octx_sb[:, ic, :], c_ps[ic][:], AF.Copy,
                    scale=zr_sb[:, ic : ic + 1],
                )
                nc.sync.dma_start(
                    out=ctx.ap()[ic * 128 : (ic + 1) * 128, :],
                    in_=octx_sb[:, ic, :],
                )
            if debug:
                nc.sync.dma_start(out=env["dbg_qT"].ap(), in_=qT16[:])
                nc.sync.dma_start(out=env["dbg_z"].ap(), in_=z_sb[:])


class _Runner:
    """Builds the Bass module once and holds a reusable jitted shard_map
    callable (mirrors concourse.bass2jax.run_bass_via_pjrt, but persistent
    so repeat calls don't re-jit/re-compile)."""

    def __init__(self, debug=False):
        import jax
        from concourse.bass2jax import install_neuronx_cc_hook, _bass_exec_p
        from jax.experimental.shard_map import shard_map
        from jax.sharding import Mesh, PartitionSpec

        self.jax = jax
        nc = bacc.Bacc(
            "TRN2", target_bir_lowering=False, debug=False,
            enable_asserts=False, num_devices=N_CORES,
            enable_partition_id=False,
        )
        _emit(nc, debug=debug)
        nc.compile()
        self.nc = nc

        install_neuronx_cc_hook()
        in_names, out_names, out_avals = [], [], []
        for alloc in nc.m.functions[0].allocations:
            if not isinstance(alloc, mybir.MemoryLocationSet):
                continue
            name = alloc.memorylocations[0].name
            if alloc.kind == "ExternalInput":
                in_names.append(name)
            elif alloc.kind == "ExternalOutput":
                out_names.append(name)
                out_avals.append(
                    jax.core.ShapedArray(
                        tuple(alloc.tensor_shape), mybir.dt.np(alloc.dtype)
                    )
                )
        assert nc.partition_id_tensor is None
        self.in_names = in_names
        self.out_names = out_names
        self.out_avals = out_avals
        n_params = len(in_names)
        n_outs = len(out_names)
        all_names = tuple(in_names + out_names)

        def _body(*args):
            outs = _bass_exec_p.bind(
                *args,
                out_avals=tuple(out_avals),
                in_names=all_names,
                out_names=tuple(out_names),
                lowering_input_output_aliases=(),
                sim_require_finite=True,
                sim_require_nnan=True,
                nc=nc,
            )
            return tuple(outs)

        devices = jax.devices()[:N_CORES]
        self.mesh = Mesh(np.asarray(devices), ("core",))
        self.pspec = PartitionSpec("core")
        in_specs = (self.pspec,) * (n_params + n_outs)
        out_specs = (self.pspec,) * n_outs
        donate = tuple(range(n_params, n_params + n_outs))
        self.sharded = jax.jit(
            shard_map(
                _body, mesh=self.mesh, in_specs=in_specs, out_specs=out_specs,
                check_rep=False,
            ),
            donate_argnums=donate,
            keep_unused=True,
        )

    def concat_inputs(self, in_maps):
        return [
            np.concatenate([np.asarray(m[name]) for m in in_maps], axis=0)
            for name in self.in_names
        ]

    def fresh_zeros(self):
        return [
            np.zeros((N_CORES * a.shape[0], *a.shape[1:]), a.dtype)
            for a in self.out_avals
        ]

    def run(self, in_maps):
        out_arrs = self.sharded(*self.concat_inputs(in_maps), *self.fresh_zeros())
        i = self.out_names.index("ctx")
        a = self.out_avals[i]
        return np.asarray(out_arrs[i]).reshape(N_CORES, *a.shape)

    def run_all(self, in_maps):
        out_arrs = self.sharded(*self.concat_inputs(in_maps), *self.fresh_zeros())
        return {
            name: np.asarray(out_arrs[i]).reshape(
                N_CORES, *self.out_avals[i].shape
            )
            for i, name in enumerate(self.out_names)
        }


_runner = None


def _get_runner():
    global _runner
    if _runner is None:
        _runner = _Runner()
    return _runner


def _make_in_maps(query, key, value, mask, Wa, Ua, scale):
    query = np.asarray(query, dtype=np.float32)
    key = np.asarray(key, dtype=np.float32)
    value = np.asarray(value, dtype=np.float32)
    mask = np.asarray(mask)
    Wa = np.ascontiguousarray(np.asarray(Wa, dtype=np.float32))
    Ua = np.ascontiguousarray(np.asarray(Ua, dtype=np.float32))
    scale = np.ascontiguousarray(np.asarray(scale, dtype=np.float32))
    cvec = np.empty(R, dtype=np.float32)
    for k, (_, _, c_sc, c_cs) in enumerate(FREQS):
        cvec[2 * k] = c_sc
        cvec[2 * k + 1] = c_cs
    wua = np.concatenate([Wa, Ua], axis=0).astype(np.float16)
    ctab = np.empty((128, R + 6), dtype=np.float32)
    ctab[:, 0:R] = scale[:, None] * cvec[None, :]
    ctab[:, R + 4] = 0.0
    ctab[:, R + 5] = PI_2
    in_maps = []
    for b in range(B):
        mb = np.where(mask[b], EXP_SHIFT, EXP_SHIFT + MASK_NEG).astype(np.float32)
        ct = ctab.copy()
        ct[:, R : R + 4] = mb.reshape(4, 128).T
        in_maps.append(
            {
                "queryT": np.ascontiguousarray(query[b].T).astype(np.float16),
                "keyT": np.ascontiguousarray(key[b].T).astype(np.float16),
                "value16": value[b].astype(np.float16),
                "wua": wua,
                "ctab": ct,
            }
        )
    return in_maps


def kernel(query, key, value, mask, Wa, Ua, scale):
    r = _get_runner()
    in_maps = _make_in_maps(query, key, value, mask, Wa, Ua, scale)
    return r.run(in_maps)

